# revision 31
# baseline (speedup 1.0000x reference)
"""Trainium2 Bass kernel: MultiHeadLatentAttention prefill (B=2, T=2048, D=2048,
H=16, HD=128, KVH=4, QL=1536, KVL=512).

Sharding: 8 cores = (batch b in {0,1}) x (kv-head group g in {0..3}).
Each core handles one batch element and the 4 q-heads of one kv head.
Host folds rms weights into up-projections, folds the (position = kv-head
index) K-rope rotation into Wkr, pre-arranges every weight into the SBUF
partition-major layout (so all DMAs are contiguous), and sums the 8 partial
outputs at the end.

v2 structure (vs v1): rms sum-of-squares accumulated on ACT(Square)+DVE adds
(no per-chunk ones-matmuls in the PE stream), rope tables pre-scaled once by
1/rms(ql), phase-3 emits score matmuls ahead of the psz/pctx consumers with a
lag so the PE never sits on the ACT exp round-trip.
"""

import numpy as np
import ml_dtypes

B, T, D = 2, 2048, 2048
H, HD, KVH = 16, 128, 4
QL, KVL = 1536, 512
G = KVH                  # core groups per batch
HPG = H // KVH           # q heads per group
NCORES = B * G
TS = 512                 # free-dim tile
NT = T // TS             # 4
DCH = D // 128           # 16
QLCH = QL // 128         # 12
CCH = KVL // 128         # 4
SCH = T // 128           # 16
EPS = 1e-6
SM_SCALE = 1.0 / 16.0    # 1/sqrt(2*HD)
BF16 = ml_dtypes.bfloat16
M_TOTAL = CCH + 1 + QLCH  # c chunks, k chunk, ql chunks

_CACHE = {}
LAST_RESULTS = None


def _build_program(reps=1, phases="1234"):
    opts = set()
    if ":" in phases:
        phases, o = phases.split(":", 1)
        opts = set(o.split(","))
    import concourse.bacc as bacc
    import concourse.tile as tile
    from concourse import mybir
    from concourse.bass import ts

    bf = mybir.dt.bfloat16
    f32 = mybir.dt.float32
    AF = mybir.ActivationFunctionType
    SWAP_MASK = [i ^ 1 for i in range(32)]

    nc = bacc.Bacc("TRN2", target_bir_lowering=False, debug=False)

    xT = nc.dram_tensor("x_T", [D, T], bf, kind="ExternalInput")
    wstream = nc.dram_tensor(
        "wstream", [M_TOTAL, 128, DCH, 128], bf, kind="ExternalInput"
    )
    wuq = nc.dram_tensor("wuq", [HPG, 128, QLCH, HD], bf, kind="ExternalInput")
    wqr = nc.dram_tensor("wqr", [HPG, 128, QLCH, HD], bf, kind="ExternalInput")
    wuk = nc.dram_tensor("wuk", [128, CCH, HD], bf, kind="ExternalInput")
    wuv = nc.dram_tensor("wuv", [128, CCH, HD], bf, kind="ExternalInput")
    wout = nc.dram_tensor("wout", [128, HPG, D], bf, kind="ExternalInput")
    ctab = nc.dram_tensor("ctab", [HD, T], bf, kind="ExternalInput")
    stab = nc.dram_tensor("stab", [HD, T], bf, kind="ExternalInput")
    outT = nc.dram_tensor("out_T", [D, T], f32, kind="ExternalOutput")

    with tile.TileContext(nc) as tc:
      for _rep in range(reps):
        with (
            tc.tile_pool(name="A", bufs=1) as A,
            tc.tile_pool(name="QLP", bufs=1) as QLP,
        ):
            c_sb = A.tile([128, CCH, T], bf)
            k_sb = A.tile([128, T], bf)
            kabs_sb = A.tile([128, T], bf)
            v_sb = A.tile([128, SCH, HD], bf)
            wuk_sb = A.tile([128, CCH, HD], bf)
            wuv_sb = A.tile([128, CCH, HD], bf)
            ones_sb = A.tile([128, 1], bf)
            sq_row = A.tile([1, T], f32)
            sc_row = A.tile([1, T], f32)
            sc_col = A.tile([128, SCH], f32)   # column form of sc (for V)
            eps_sb = A.tile([1, 1], f32)
            ql_sb = QLP.tile([128, QLCH, T], bf)
            ctabp = QLP.tile([128, T], bf)     # cos table * 1/rms(ql)
            stabp = QLP.tile([128, T], bf)     # sin table * 1/rms(ql)
            sqq_bc = QLP.tile([128, T], f32)   # broadcast of 1/rms(ql)

            # ---------------- phase 1: latent projections from x -------------
            with (
                tc.tile_pool(name="PH1", bufs=1) as P1,
                tc.tile_pool(name="P1S", bufs=3) as P1S,
                tc.tile_pool(name="SQP", bufs=2) as SQP,
                tc.tile_pool(name="DRS", bufs=1, space="DRAM") as DRS,
                tc.tile_pool(name="PP1", bufs=4 if "dreuse" in opts else 3, space="PSUM") as PP1,
                tc.tile_pool(name="PZ1", bufs=2, space="PSUM") as PZ1,
            ):
                scc_bc = P1.tile([128, T], f32)    # broadcast of 1/rms(c)
                acc_c = P1.tile([128, T], bf)      # per-token sumsq partials (c)
                acc_q = P1.tile([128, T], bf)      # per-token sumsq partials (ql)
                xT_r = xT.ap().rearrange("(c p) t -> c p t", p=128)
                x_sb = []
                for d in range(DCH):
                    xd = P1.tile([128, T], bf, tag=f"x{d}", name=f"x{d}")
                    nc.sync.dma_start(out=xd, in_=xT_r[d])
                    x_sb.append(xd)

                w_ts = []
                for m in range(M_TOTAL):
                    w_t = P1S.tile([128, DCH, 128], bf, tag="wstream")
                    nc.sync.dma_start(out=w_t, in_=wstream.ap()[m])
                    w_ts.append(w_t)
                    if m == 1:
                        # small constants after the first two weight slices
                        nc.vector.memset(ones_sb, 1.0)
                        nc.vector.memset(eps_sb, EPS)
                        nc.sync.dma_start(out=wuk_sb, in_=wuk.ap())
                        nc.sync.dma_start(out=wuv_sb, in_=wuv.ap())

                DL = list(range(0, DCH, 2)) if "halfd" in opts else list(range(DCH))
                for m in range(M_TOTAL):
                    w_t = w_ts[m]
                    is_c = m < CCH
                    is_k = m == CCH
                    if "dreuse" in opts:
                        # d-outer: one LDWEIGHTS per (m, d), reused across the
                        # 4 token tiles accumulating into 4 PSUM banks
                        ps4 = [
                            PP1.tile([128, TS], f32, tag="pp", name=f"pp{i}")
                            for i in range(NT)
                        ]
                        for di, d in enumerate(DL):
                            for t in range(NT):
                                nc.tensor.matmul(
                                    ps4[t],
                                    lhsT=w_t[:, d, :],
                                    rhs=x_sb[d][:, ts(t, TS)],
                                    start=(di == 0),
                                    stop=(di == len(DL) - 1),
                                )
                    for t in range(NT):
                        if "dreuse" in opts:
                            ps = ps4[t]
                        else:
                            ps = PP1.tile([128, TS], f32, tag="pp")
                            for di, d in enumerate(DL):
                                nc.tensor.matmul(
                                    ps,
                                    lhsT=w_t[:, d, :],
                                    rhs=x_sb[d][:, ts(t, TS)],
                                    start=(di == 0),
                                    stop=(di == len(DL) - 1),
                                )
                        if is_c:
                            dst = c_sb[:, m, ts(t, TS)]
                        elif is_k:
                            dst = k_sb[:, ts(t, TS)]
                        else:
                            dst = ql_sb[:, m - CCH - 1, ts(t, TS)]
                        nc.vector.tensor_copy(dst, ps)
                        if is_k:
                            continue
                        # rms stats off the PE stream: square on ACT,
                        # accumulate on DVE
                        acc = acc_c if is_c else acc_q
                        mi = m if is_c else m - CCH - 1
                        nch = CCH if is_c else QLCH
                        if mi == 0:
                            nc.scalar.activation(
                                acc[:, ts(t, TS)], ps, AF.Square
                            )
                        else:
                            sqt = SQP.tile([128, TS], bf, tag="sq")
                            nc.scalar.activation(sqt, ps, AF.Square)
                            nc.vector.tensor_add(
                                acc[:, ts(t, TS)], acc[:, ts(t, TS)], sqt
                            )
                        if mi == nch - 1:
                            # partition-sum of sumsq, then sqrt(mean + eps)
                            psz = PZ1.tile([1, TS], f32, tag="pz")
                            nc.tensor.matmul(
                                psz, lhsT=ones_sb, rhs=acc[:, ts(t, TS)],
                                start=True, stop=True,
                            )
                            row = sc_row if is_c else sq_row
                            nrm = KVL if is_c else QL
                            nc.scalar.activation(
                                row[0:1, ts(t, TS)],
                                psz,
                                AF.Sqrt,
                                bias=eps_sb[0:1, 0:1],
                                scale=1.0 / nrm,
                            )
                    # scale chains as soon as each row completes
                    if m == CCH - 1:
                        nc.vector.reciprocal(sc_row[0:1, :], sc_row[0:1, :])
                        nc.gpsimd.partition_broadcast(scc_bc, sc_row[0:1, :])
                        # column form of sc via DRAM round-trip
                        dr = DRS.tile([1, T], f32)
                        nc.sync.dma_start(out=dr, in_=sc_row[0:1, :])
                        nc.sync.dma_start(
                            out=sc_col,
                            in_=dr[:, :].rearrange("o (s p) -> (o p) s", p=128),
                        )
                    if m == M_TOTAL - 1:
                        nc.vector.reciprocal(sq_row[0:1, :], sq_row[0:1, :])
                        nc.gpsimd.partition_broadcast(sqq_bc, sq_row[0:1, :])

                # prescaled rope tables (fold 1/rms(ql) into cos/sin once,
                # in place)
                nc.sync.dma_start(out=ctabp, in_=ctab.ap())
                nc.sync.dma_start(out=stabp, in_=stab.ap())
                for t in range(NT):
                    nc.vector.tensor_mul(
                        ctabp[:, ts(t, TS)], ctabp[:, ts(t, TS)],
                        sqq_bc[:, ts(t, TS)],
                    )
                    nc.vector.tensor_mul(
                        stabp[:, ts(t, TS)], stabp[:, ts(t, TS)],
                        sqq_bc[:, ts(t, TS)],
                    )

                # Kabs (absorbed Wuk) and token-major V from raw c + epilogue
                for t in range(NT):
                    ps = PP1.tile([128, TS], f32, tag="pp")
                    for cc in range(CCH):
                        nc.tensor.matmul(
                            ps,
                            lhsT=wuk_sb[:, cc, :],
                            rhs=c_sb[:, cc, ts(t, TS)],
                            start=(cc == 0),
                            stop=(cc == CCH - 1),
                        )
                    nc.vector.tensor_mul(
                        kabs_sb[:, ts(t, TS)], ps, scc_bc[:, ts(t, TS)]
                    )
                for s in range(SCH):
                    ps = PP1.tile([128, TS], f32, tag="pp")
                    for cc in range(CCH):
                        nc.tensor.matmul(
                            ps[:, 0:HD],
                            lhsT=c_sb[:, cc, s * 128:(s + 1) * 128],
                            rhs=wuv_sb[:, cc, :],
                            start=(cc == 0),
                            stop=(cc == CCH - 1),
                        )
                    nc.vector.tensor_scalar_mul(
                        v_sb[:, s, :], ps[:, 0:HD], sc_col[:, s:s + 1]
                    )

            if phases == "1":
                with tc.tile_pool(name="DBG", bufs=1) as DBGO:
                    dbg = DBGO.tile([128, 3, T], f32)
                    for t in range(NT):
                        nc.vector.tensor_copy(
                            dbg[:, 0, ts(t, TS)], kabs_sb[:, ts(t, TS)]
                        )
                        nc.vector.tensor_copy(
                            dbg[:, 1, ts(t, TS)], k_sb[:, ts(t, TS)]
                        )
                        nc.vector.tensor_copy(
                            dbg[:, 2, t * TS:t * TS + HD], v_sb[:, 4 * t, :]
                        )
                    nc.gpsimd.dma_start(
                        out=outT.ap()[0:384, :].rearrange(
                            "(c p) t -> p c t", p=128
                        ),
                        in_=dbg,
                    )
                continue

            # ---------------- phases 2+3: per-head Q/Qr + attention ----------
            with tc.tile_pool(name="P3B", bufs=1) as P3B:
                # per-(head, j) ctx tiles so phase 4 can start on early tiles
                ctx_sb = [
                    [
                        P3B.tile([128, TS], bf, tag=f"ctx{h}_{j}", name=f"ctx{h}_{j}")
                        for j in range(NT)
                    ]
                    for h in range(HPG)
                ]
                wout_sb = P3B.tile([128, HPG, T], bf)
                masks_sb = P3B.tile([128, 4, TS], bf)
                for r in range(4):
                    nc.vector.memset(masks_sb[:, r, :], 1.0)
                    nc.gpsimd.affine_select(
                        out=masks_sb[:, r, :],
                        in_=masks_sb[:, r, :],
                        pattern=[[1, TS]],
                        compare_op=mybir.AluOpType.is_ge,
                        fill=0.0,
                        base=-(128 * r),
                        channel_multiplier=-1,
                    )
                with (
                    tc.tile_pool(name="P3S", bufs=2) as P3S,
                    tc.tile_pool(name="EP", bufs=8) as EP,
                    tc.tile_pool(name="ESP", bufs=2) as ESP,
                    tc.tile_pool(name="TMPP", bufs=4) as TMPP,
                    tc.tile_pool(name="ZR", bufs=3) as ZR,
                    tc.tile_pool(name="PQK", bufs=2, space="PSUM") as PQK,
                    tc.tile_pool(name="PSC", bufs=3, space="PSUM") as PSC,
                    tc.tile_pool(name="PCT", bufs=2, space="PSUM") as PCT,
                    tc.tile_pool(name="PZ3", bufs=1, space="PSUM") as PZ3,
                ):
                    for h in range(HPG):
                        wuq_t = P3S.tile([128, QLCH, HD], bf, tag="wuq")
                        wqr_t = P3S.tile([128, QLCH, HD], bf, tag="wqr")
                        nc.sync.dma_start(out=wuq_t, in_=wuq.ap()[h])
                        nc.sync.dma_start(out=wqr_t, in_=wqr.ap()[h])
                        if h == 0:
                            # prefetch phase-4 weights behind head-0 weights
                            nc.sync.dma_start(out=wout_sb, in_=wout.ap())
                        qh_sb = P3S.tile([128, T], bf, tag="qh")
                        qrh_sb = P3S.tile([128, T], bf, tag="qrh")
                        for t in range(NT):
                            psq = PQK.tile([128, TS], f32, tag="pqk")
                            for m in range(QLCH):
                                nc.tensor.matmul(
                                    psq,
                                    lhsT=wuq_t[:, m, :],
                                    rhs=ql_sb[:, m, ts(t, TS)],
                                    start=(m == 0),
                                    stop=(m == QLCH - 1),
                                )
                            nc.vector.tensor_mul(
                                qh_sb[:, ts(t, TS)], psq, sqq_bc[:, ts(t, TS)]
                            )
                            psr = PQK.tile([128, TS], f32, tag="pqk")
                            for m in range(QLCH):
                                nc.tensor.matmul(
                                    psr,
                                    lhsT=wqr_t[:, m, :],
                                    rhs=ql_sb[:, m, ts(t, TS)],
                                    start=(m == 0),
                                    stop=(m == QLCH - 1),
                                )
                            # rope with prescaled tables:
                            # qrh = psr*ctabp + pairswap(psr)*stabp
                            tsw = TMPP.tile([128, TS], f32, tag="tmp")
                            nc.vector.stream_shuffle(tsw, psr, SWAP_MASK)
                            t1 = TMPP.tile([128, TS], f32, tag="tmp")
                            nc.vector.tensor_mul(t1, psr, ctabp[:, ts(t, TS)])
                            t2 = TMPP.tile([128, TS], f32, tag="tmp")
                            nc.vector.tensor_mul(t2, tsw, stabp[:, ts(t, TS)])
                            nc.vector.tensor_add(qrh_sb[:, ts(t, TS)], t1, t2)

                        if "3" not in phases:
                            for t in range(NT):
                                dq = TMPP.tile([128, TS], f32, tag="dbg2")
                                nc.vector.tensor_copy(dq, qh_sb[:, ts(t, TS)])
                                nc.gpsimd.dma_start(
                                    out=outT.ap()[
                                        h * 256:h * 256 + 128, ts(t, TS)
                                    ],
                                    in_=dq,
                                )
                                dr2 = TMPP.tile([128, TS], f32, tag="dbg2")
                                nc.vector.tensor_copy(dr2, qrh_sb[:, ts(t, TS)])
                                nc.gpsimd.dma_start(
                                    out=outT.ap()[
                                        h * 256 + 128:h * 256 + 256, ts(t, TS)
                                    ],
                                    in_=dr2,
                                )
                            continue

                        for j in range(NT):
                            n_s = 4 * (j + 1)
                            LAG = 5
                            pctx = PCT.tile([128, TS], f32, tag="pct")
                            esum = ESP.tile([128, TS], bf, tag="esum")
                            e_ts = [None] * n_s

                            def consume(s):
                                nc.tensor.matmul(
                                    pctx,
                                    lhsT=v_sb[:, s, :],
                                    rhs=e_ts[s],
                                    start=(s == 0),
                                    stop=(s == n_s - 1),
                                )

                            for s in range(n_s):
                                pss = PSC.tile([128, TS], f32, tag="psc")
                                nc.tensor.matmul(
                                    pss,
                                    lhsT=kabs_sb[:, s * 128:(s + 1) * 128],
                                    rhs=qh_sb[:, ts(j, TS)],
                                    start=True,
                                    stop=False,
                                )
                                nc.tensor.matmul(
                                    pss,
                                    lhsT=k_sb[:, s * 128:(s + 1) * 128],
                                    rhs=qrh_sb[:, ts(j, TS)],
                                    start=False,
                                    stop=True,
                                )
                                e_t = EP.tile([128, TS], bf, tag="e")
                                nc.scalar.activation(e_t, pss, AF.Exp, scale=SM_SCALE)
                                if s >= 4 * j:
                                    nc.vector.tensor_mul(
                                        e_t, e_t, masks_sb[:, s - 4 * j, :]
                                    )
                                e_ts[s] = e_t
                                # softmax denominator accumulated on DVE
                                if s == 0:
                                    nc.vector.tensor_copy(esum, e_t)
                                else:
                                    nc.vector.tensor_add(esum, esum, e_t)
                                if s >= LAG:
                                    consume(s - LAG)
                            for s in range(max(0, n_s - LAG), n_s):
                                consume(s)
                            psz = PZ3.tile([1, TS], f32, tag="pz3")
                            nc.tensor.matmul(
                                psz, lhsT=ones_sb, rhs=esum,
                                start=True, stop=True,
                            )

                            zrow = ZR.tile([1, TS], f32, tag="zrow")
                            nc.vector.tensor_copy(zrow, psz)
                            zinv = ZR.tile([1, TS], f32, tag="zrow")
                            nc.vector.reciprocal(zinv, zrow)
                            zbc = TMPP.tile([128, TS], f32, tag="zbc")
                            nc.gpsimd.partition_broadcast(zbc, zinv[0:1, :])
                            nc.vector.tensor_mul(ctx_sb[h][j], pctx, zbc)

                if "4" not in phases:
                    if "3" in phases:
                        with tc.tile_pool(name="DBG3", bufs=4) as DBG3:
                            for h2 in range(HPG):
                                for j2 in range(NT):
                                    dc = DBG3.tile([128, TS], f32, tag="dbg3")
                                    nc.vector.tensor_copy(dc, ctx_sb[h2][j2])
                                    nc.gpsimd.dma_start(
                                        out=outT.ap()[
                                            h2 * 128:(h2 + 1) * 128,
                                            ts(j2, TS),
                                        ],
                                        in_=dc,
                                    )
                    continue

                # ---------------- phase 4: output projection -----------------
                with (
                    tc.tile_pool(name="P4", bufs=6) as P4,
                    tc.tile_pool(name="PP4", bufs=3, space="PSUM") as PP4,
                ):
                    for e in range(DCH):
                        for t in range(NT):
                            ps = PP4.tile([128, TS], f32, tag="pp4")
                            for q in range(HPG):
                                nc.tensor.matmul(
                                    ps,
                                    lhsT=wout_sb[:, q, e * 128:(e + 1) * 128],
                                    rhs=ctx_sb[q][t],
                                    start=(q == 0),
                                    stop=(q == HPG - 1),
                                )
                            o_t = P4.tile([128, TS], f32, tag="ot")
                            nc.vector.tensor_copy(o_t, ps)
                            nc.sync.dma_start(
                                out=outT.ap()[
                                    e * 128:(e + 1) * 128, ts(t, TS)
                                ],
                                in_=o_t,
                            )

    nc.compile()
    return nc


def _get_program():
    if "nc" not in _CACHE:
        _CACHE["nc"] = _build_program()
    return _CACHE["nc"]


def _chunk_pm(block):
    """[D?, 128cols] weight block -> partition-major [128, D//128, 128]."""
    d = block.shape[0]
    return np.ascontiguousarray(
        block.reshape(d // 128, 128, block.shape[1]).transpose(1, 0, 2)
    )


def _host_prep(inputs):
    """Fold weights on the host and build the 8 per-core input maps."""
    x = np.asarray(inputs["x"], np.float32)
    Wdq = np.asarray(inputs["Wdq"], np.float32)
    qw = np.asarray(inputs["q_norm_w"], np.float32)
    Wuq = np.asarray(inputs["Wuq"], np.float32) * qw[None, :]
    Wqr = np.asarray(inputs["Wqr"], np.float32) * qw[None, :]
    Wdkv = np.asarray(inputs["Wdkv"], np.float32)
    kvw = np.asarray(inputs["kv_norm_w"], np.float32)
    Wuk = np.asarray(inputs["Wuk"], np.float32) * kvw[None, :]
    Wuv = np.asarray(inputs["Wuv"], np.float32) * kvw[None, :]
    Wkr = np.asarray(inputs["Wkr"], np.float32)
    Wout = np.asarray(inputs["Wout"], np.float32)

    inv = 1.0 / (10000.0 ** (np.arange(0, HD, 2, dtype=np.float32) / HD))
    f = np.arange(T, dtype=np.float32)[None, :] * inv[:, None]   # [64, T]
    cosT, sinT = np.cos(f), np.sin(f)
    Ctab = np.repeat(cosT, 2, axis=0)                            # [128, T]
    Stab = np.repeat(sinT, 2, axis=0)
    Stab[0::2, :] *= -1.0                                        # pair-swap sign

    fH = np.arange(KVH, dtype=np.float32)[None, :] * inv[:, None]  # [64, KVH]
    cosH, sinH = np.cos(fH), np.sin(fH)

    def bft(a):
        return np.ascontiguousarray(a).astype(BF16)

    wdqT = Wdq.T            # [D, QL]
    wdkvT = Wdkv.T          # [D, KVL]
    # shared wstream chunks (c then placeholder-k then ql)
    base_chunks = [
        _chunk_pm(wdkvT[:, m * 128:(m + 1) * 128]) for m in range(CCH)
    ]
    ql_chunks = [
        _chunk_pm(wdqT[:, m * 128:(m + 1) * 128]) for m in range(QLCH)
    ]
    wuk_pm = bft(_chunk_pm(Wuk.T))                  # [128, CCH, HD]
    ctab_b = bft(Ctab)
    stab_b = bft(Stab)

    in_maps = []
    for b in range(B):
        x_T = bft(x[b].T)
        for g in range(G):
            # fold K-rope (fixed rotation per kv-head index) into Wkr
            Wkr_g = Wkr[g * HD:(g + 1) * HD, :]
            we, wo = Wkr_g[0::2, :], Wkr_g[1::2, :]
            c_g, s_g = cosH[:, g][:, None], sinH[:, g][:, None]
            Wkr_eff = np.empty_like(Wkr_g)
            Wkr_eff[0::2, :] = we * c_g - wo * s_g
            Wkr_eff[1::2, :] = we * s_g + wo * c_g

            wstream_np = np.stack(
                base_chunks + [_chunk_pm(Wkr_eff.T)] + ql_chunks
            )  # [M_TOTAL, 128, DCH, 128]

            Wuq_g = Wuq[g * HPG * HD:(g + 1) * HPG * HD].T   # [QL, 512]
            Wqr_g = Wqr[g * HPG * HD:(g + 1) * HPG * HD].T
            wuq_np = np.stack(
                [_chunk_pm(Wuq_g[:, h * HD:(h + 1) * HD]) for h in range(HPG)]
            )  # [HPG, 128, QLCH, HD]
            wqr_np = np.stack(
                [_chunk_pm(Wqr_g[:, h * HD:(h + 1) * HD]) for h in range(HPG)]
            )

            in_maps.append(
                dict(
                    x_T=x_T,
                    wstream=bft(wstream_np),
                    wuq=bft(wuq_np),
                    wqr=bft(wqr_np),
                    wuk=wuk_pm,
                    wuv=bft(_chunk_pm(Wuv[g * HD:(g + 1) * HD].T)),
                    wout=bft(_chunk_pm(Wout[:, g * HPG * HD:(g + 1) * HPG * HD].T)),
                    ctab=ctab_b,
                    stab=stab_b,
                )
            )
    return in_maps


def kernel(**inputs):
    global LAST_RESULTS
    from concourse import bass_utils

    nc = _get_program()
    in_maps = _host_prep(inputs)
    res = bass_utils.run_bass_kernel_spmd(
        nc, in_maps, core_ids=list(range(NCORES))
    )
    LAST_RESULTS = res
    out = np.zeros((B, T, D), np.float32)
    for i, r in enumerate(res.results):
        out[i // G] += r["out_T"].T
    return out


# revision 37
# speedup vs baseline: 1.3292x; 1.3292x over previous
"""Trainium2 Bass kernel: MultiHeadLatentAttention prefill (B=2, T=2048, D=2048,
H=16, HD=128, KVH=4, QL=1536, KVL=512).

Sharding: 8 cores = (batch b in {0,1}) x (kv-head group g in {0..3}).
Phases 2-4 (Q/Qr, attention, out-proj) are head-sharded as before: each core
handles the 4 q-heads of one kv group for the full sequence. Phase 1 (the
latent projections ql/c, which v2 replicated 4x per batch) is token-sharded:
each core projects only its 512-token quarter, then a 4-core AllGather
exchanges (ql, c, inv-rms rows); the K-rope projection (per-group weights) is
computed locally over the full sequence during the collective's wire time.
Host folds rms weights into up-projections, folds the (position = kv-head
index) K-rope rotation into Wkr, pre-arranges weights partition-major (all
DMAs contiguous), and sums the 8 partial outputs.
"""

import numpy as np
import ml_dtypes

B, T, D = 2, 2048, 2048
H, HD, KVH = 16, 128, 4
QL, KVL = 1536, 512
G = KVH                  # core groups per batch
HPG = H // KVH           # q heads per group
NCORES = B * G
TS = 512                 # free-dim tile
NT = T // TS             # 4
DCH = D // 128           # 16
QLCH = QL // 128         # 12
CCH = KVL // 128         # 4
SCH = T // 128           # 16
EPS = 1e-6
SM_SCALE = 1.0 / 16.0    # 1/sqrt(2*HD)
BF16 = ml_dtypes.bfloat16
M_LOC = QLCH + CCH        # ql chunks then c chunks (local-quarter stream)
GROWS = 128               # gather rows: q-side inv-rms row, chunk-padded

_CACHE = {}
LAST_RESULTS = None


def _build_program(reps=1, phases="1234"):
    opts = set()
    if ":" in phases:
        phases, o = phases.split(":", 1)
        opts = set(o.split(","))
    import concourse.bacc as bacc
    import concourse.tile as tile
    from concourse import mybir
    from concourse.bass import ts

    bf = mybir.dt.bfloat16
    f32 = mybir.dt.float32
    AF = mybir.ActivationFunctionType
    SWAP_MASK = [i ^ 1 for i in range(32)]

    nc = bacc.Bacc("TRN2", target_bir_lowering=False, debug=False)

    xT = nc.dram_tensor("x_T", [D, T], bf, kind="ExternalInput")
    xL = nc.dram_tensor("xL_T", [D, TS], bf, kind="ExternalInput")
    wloc = nc.dram_tensor(
        "wloc", [M_LOC, 128, DCH, 128], bf, kind="ExternalInput"
    )
    wkr = nc.dram_tensor("wkr", [128, DCH, 128], bf, kind="ExternalInput")
    wuq = nc.dram_tensor("wuq", [HPG, 128, DCH, HD], bf, kind="ExternalInput")
    wqr = nc.dram_tensor("wqr", [HPG, 128, DCH, HD], bf, kind="ExternalInput")
    wuk = nc.dram_tensor("wuk", [128, CCH, HD], bf, kind="ExternalInput")
    wuv = nc.dram_tensor("wuv", [128, CCH, HD], bf, kind="ExternalInput")
    wout = nc.dram_tensor("wout", [128, HPG, D], bf, kind="ExternalInput")
    ctab = nc.dram_tensor("ctab", [HD, T], bf, kind="ExternalInput")
    stab = nc.dram_tensor("stab", [HD, T], bf, kind="ExternalInput")
    outT = nc.dram_tensor("out_T", [D, T], f32, kind="ExternalOutput")
    RG = [[0, 1, 2, 3], [4, 5, 6, 7]]

    with tile.TileContext(nc) as tc:
      for _rep in range(reps):
        with (
            tc.tile_pool(name="A", bufs=1) as A,
            tc.tile_pool(name="QLP", bufs=1) as QLP,
        ):
            c_sb = A.tile([128, CCH, T], bf)
            k_sb = A.tile([128, T], bf)
            kabs_sb = A.tile([128, T], bf)
            v_sb = A.tile([128, SCH, HD], bf)
            wuk_sb = A.tile([128, CCH, HD], bf)
            wuv_sb = A.tile([128, CCH, HD], bf)
            ones_sb = A.tile([128, 1], bf)
            sc_col = A.tile([128, SCH], f32)   # column form of 1/rms(c) (for V)
            x_res = QLP.tile([128, DCH, T], bf)  # resident x (phase-2 rhs + K + c)
            ctabp = QLP.tile([128, T], bf)     # cos table * 1/rms(ql)
            stabp = QLP.tile([128, T], bf)     # sin table * 1/rms(ql)
            sqq_bc = QLP.tile([128, T], f32)   # broadcast of 1/rms(ql)

            # ------------- phase 1: token-sharded latent projections ---------
            with (
                tc.tile_pool(name="PH1", bufs=1) as P1,
                tc.tile_pool(name="P1S", bufs=3) as P1S,
                tc.tile_pool(name="XKS", bufs=3) as XKS,
                tc.tile_pool(name="SQP", bufs=2) as SQP,
                tc.tile_pool(name="CCD", bufs=1, space="DRAM") as CCD,
                tc.tile_pool(name="PP1", bufs=4, space="PSUM") as PP1,
                tc.tile_pool(name="PZ1", bufs=2, space="PSUM") as PZ1,
            ):
                scc_bc = P1.tile([128, T], f32)    # broadcast of 1/rms(c)
                acc_q = P1.tile([128, TS], bf)     # local sumsq partials (ql)
                rowq_bf = P1.tile([1, T], bf)      # inv-rms row (q side)
                rowq_f = P1.tile([1, T], f32)
                rowc_f = P1.tile([1, T], f32)
                eps_sb = P1.tile([1, 1], f32)
                xL_r = xL.ap().rearrange("(c p) t -> c p t", p=128)
                xT_r = xT.ap().rearrange("(c p) t -> c p t", p=128)
                xl_sb = []
                for d in range(DCH):
                    xd = P1.tile([128, TS], bf, tag=f"xl{d}", name=f"xl{d}")
                    nc.sync.dma_start(out=xd, in_=xL_r[d])
                    xl_sb.append(xd)
                for d in range(DCH):
                    nc.sync.dma_start(out=x_res[:, d, :], in_=xT_r[d])

                w_ts = []
                for m in range(M_LOC):
                    w_t = P1S.tile([128, DCH, 128], bf, tag="wstream")
                    nc.sync.dma_start(out=w_t, in_=wloc.ap()[m])
                    w_ts.append(w_t)
                    if m == 1:
                        nc.vector.memset(ones_sb, 1.0)
                        nc.vector.memset(eps_sb, EPS)
                        nc.sync.dma_start(out=wuk_sb, in_=wuk.ap())
                        nc.sync.dma_start(out=wuv_sb, in_=wuv.ap())

                # local ql chunks: only the rms statistic is needed
                for m in range(QLCH):
                    w_t = w_ts[m]
                    ps = PP1.tile([128, TS], f32, tag="pp")
                    for d in range(DCH):
                        nc.tensor.matmul(
                            ps,
                            lhsT=w_t[:, d, :],
                            rhs=xl_sb[d],
                            start=(d == 0),
                            stop=(d == DCH - 1),
                        )
                    if m == 0:
                        nc.scalar.activation(acc_q, ps, AF.Square)
                    else:
                        sqt = SQP.tile([128, TS], bf, tag="sq")
                        nc.scalar.activation(sqt, ps, AF.Square)
                        nc.vector.tensor_add(acc_q, acc_q, sqt)
                    if m == QLCH - 1:
                        psz = PZ1.tile([1, TS], f32, tag="pz")
                        nc.tensor.matmul(
                            psz, lhsT=ones_sb, rhs=acc_q, start=True, stop=True
                        )
                        nc.scalar.activation(
                            rowq_f[0:1, 0:TS],
                            psz,
                            AF.Sqrt,
                            bias=eps_sb[0:1, 0:1],
                            scale=1.0 / QL,
                        )
                        nc.vector.reciprocal(
                            rowq_f[0:1, 0:TS], rowq_f[0:1, 0:TS]
                        )
                        nc.vector.tensor_copy(
                            rowq_bf[0:1, 0:TS], rowq_f[0:1, 0:TS]
                        )

                # c computed over the FULL sequence from resident x
                # (replicated across the group; no c gather)
                acc_cf = P1.tile([128, NT, TS], bf)
                for m in range(CCH):
                    w_t = w_ts[QLCH + m]
                    for t in range(NT):
                        ps = PP1.tile([128, TS], f32, tag="pp")
                        for d in range(DCH):
                            nc.tensor.matmul(
                                ps,
                                lhsT=w_t[:, d, :],
                                rhs=x_res[:, d, ts(t, TS)],
                                start=(d == 0),
                                stop=(d == DCH - 1),
                            )
                        nc.vector.tensor_copy(c_sb[:, m, ts(t, TS)], ps)
                        if m == 0:
                            nc.scalar.activation(
                                acc_cf[:, t, :], ps, AF.Square
                            )
                        else:
                            sqt = SQP.tile([128, TS], bf, tag="sq")
                            nc.scalar.activation(sqt, ps, AF.Square)
                            nc.vector.tensor_add(
                                acc_cf[:, t, :], acc_cf[:, t, :], sqt
                            )
                        if m == CCH - 1:
                            psz = PZ1.tile([1, TS], f32, tag="pz")
                            nc.tensor.matmul(
                                psz, lhsT=ones_sb, rhs=acc_cf[:, t, :],
                                start=True, stop=True,
                            )
                            nc.scalar.activation(
                                rowc_f[0:1, ts(t, TS)],
                                psz,
                                AF.Sqrt,
                                bias=eps_sb[0:1, 0:1],
                                scale=1.0 / KVL,
                            )
                nc.vector.reciprocal(rowc_f[0:1, :], rowc_f[0:1, :])
                # column form of 1/rms(c) via DRAM round-trip (for V scaling)
                drs = CCD.tile([1, T], f32)
                nc.sync.dma_start(out=drs, in_=rowc_f[0:1, :])
                nc.sync.dma_start(
                    out=sc_col,
                    in_=drs[:, :].rearrange("o (s p) -> (o p) s", p=128),
                )

                # pack local quarter -> internal DRAM, AllGather across the
                # 4 cores of this batch
                cin = CCD.tile([GROWS, TS], bf)
                cout = CCD.tile([4 * GROWS, TS], bf)
                # row 0 carries the data; rows 1..127 are chunk padding so the
                # collective payload stays above the racy tiny-transfer regime
                nc.sync.dma_start(
                    out=cin[0:1, :], in_=rowq_bf[0:1, 0:TS]
                )
                nc.sync.dma_start(
                    out=cin[1:2, :], in_=rowq_bf[0:1, 0:TS]
                )
                nc.gpsimd.collective_compute(
                    "AllGather",
                    mybir.AluOpType.bypass,
                    ins=[cin[:]],
                    outs=[cout[:]],
                    replica_groups=RG,
                )

                # K projection (per-group weights, full sequence) overlaps the
                # collective: weights ldweights-reused across 4 token tiles,
                # x streamed in chunks
                wk_t = P1.tile([128, DCH, 128], bf)
                nc.sync.dma_start(out=wk_t, in_=wkr.ap())
                psk = [
                    PP1.tile([128, TS], f32, tag="pp", name=f"psk{t}")
                    for t in range(NT)
                ]
                for d in range(DCH):
                    for t in range(NT):
                        nc.tensor.matmul(
                            psk[t],
                            lhsT=wk_t[:, d, :],
                            rhs=x_res[:, d, ts(t, TS)],
                            start=(d == 0),
                            stop=(d == DCH - 1),
                        )
                for t in range(NT):
                    nc.vector.tensor_copy(k_sb[:, ts(t, TS)], psk[t])

                # unpack gathered ql / c / rows
                for q in range(4):
                    nc.sync.dma_start(
                        out=rowq_bf[0:1, q * TS:(q + 1) * TS],
                        in_=cout[q * GROWS:q * GROWS + 1, :],
                    )

                # gathered q row -> f32 -> partition broadcasts
                nc.vector.tensor_copy(rowq_f[0:1, :], rowq_bf[0:1, :])
                nc.gpsimd.partition_broadcast(sqq_bc, rowq_f[0:1, :])
                nc.gpsimd.partition_broadcast(scc_bc, rowc_f[0:1, :])

                # prescaled rope tables (fold 1/rms(ql) into cos/sin in place)
                nc.sync.dma_start(out=ctabp, in_=ctab.ap())
                nc.sync.dma_start(out=stabp, in_=stab.ap())
                for t in range(NT):
                    nc.vector.tensor_mul(
                        ctabp[:, ts(t, TS)], ctabp[:, ts(t, TS)],
                        sqq_bc[:, ts(t, TS)],
                    )
                    nc.vector.tensor_mul(
                        stabp[:, ts(t, TS)], stabp[:, ts(t, TS)],
                        sqq_bc[:, ts(t, TS)],
                    )

                # Kabs (absorbed Wuk) and token-major V from gathered c
                for t in range(NT):
                    ps = PP1.tile([128, TS], f32, tag="pp")
                    for cc in range(CCH):
                        nc.tensor.matmul(
                            ps,
                            lhsT=wuk_sb[:, cc, :],
                            rhs=c_sb[:, cc, ts(t, TS)],
                            start=(cc == 0),
                            stop=(cc == CCH - 1),
                        )
                    nc.vector.tensor_mul(
                        kabs_sb[:, ts(t, TS)], ps, scc_bc[:, ts(t, TS)]
                    )
                for s in range(SCH):
                    ps = PP1.tile([128, TS], f32, tag="pp")
                    for cc in range(CCH):
                        nc.tensor.matmul(
                            ps[:, 0:HD],
                            lhsT=c_sb[:, cc, s * 128:(s + 1) * 128],
                            rhs=wuv_sb[:, cc, :],
                            start=(cc == 0),
                            stop=(cc == CCH - 1),
                        )
                    nc.vector.tensor_scalar_mul(
                        v_sb[:, s, :], ps[:, 0:HD], sc_col[:, s:s + 1]
                    )

            # ---------------- phases 2+3: per-head Q/Qr + attention ----------
            with tc.tile_pool(name="P3B", bufs=1) as P3B:
                ctx_sb = [
                    [
                        P3B.tile([128, TS], bf, tag=f"ctx{h}_{j}", name=f"ctx{h}_{j}")
                        for j in range(NT)
                    ]
                    for h in range(HPG)
                ]
                wout_sb = P3B.tile([128, HPG, T], bf)
                masks_sb = P3B.tile([128, 4, TS], bf)
                for r in range(4):
                    nc.vector.memset(masks_sb[:, r, :], 1.0)
                    nc.gpsimd.affine_select(
                        out=masks_sb[:, r, :],
                        in_=masks_sb[:, r, :],
                        pattern=[[1, TS]],
                        compare_op=mybir.AluOpType.is_ge,
                        fill=0.0,
                        base=-(128 * r),
                        channel_multiplier=-1,
                    )
                with (
                    tc.tile_pool(name="P3S", bufs=2) as P3S,
                    tc.tile_pool(name="EP", bufs=8) as EP,
                    tc.tile_pool(name="ESP", bufs=2) as ESP,
                    tc.tile_pool(name="TMPP", bufs=3) as TMPP,
                    tc.tile_pool(name="ZR", bufs=2) as ZR,
                    tc.tile_pool(name="PQK", bufs=2, space="PSUM") as PQK,
                    tc.tile_pool(name="PSC", bufs=3, space="PSUM") as PSC,
                    tc.tile_pool(name="PCT", bufs=2, space="PSUM") as PCT,
                    tc.tile_pool(name="PZ3", bufs=1, space="PSUM") as PZ3,
                ):
                    for h in range(HPG):
                        wuq_t = P3S.tile([128, DCH, HD], bf, tag="wuq")
                        wqr_t = P3S.tile([128, DCH, HD], bf, tag="wqr")
                        nc.sync.dma_start(out=wuq_t, in_=wuq.ap()[h])
                        nc.sync.dma_start(out=wqr_t, in_=wqr.ap()[h])
                        if h == 0:
                            nc.sync.dma_start(out=wout_sb, in_=wout.ap())
                        qh_sb = P3S.tile([128, T], bf, tag="qh")
                        qrh_sb = P3S.tile([128, T], bf, tag="qrh")
                        for t in range(NT):
                            psq = PQK.tile([128, TS], f32, tag="pqk")
                            for m in range(DCH):
                                nc.tensor.matmul(
                                    psq,
                                    lhsT=wuq_t[:, m, :],
                                    rhs=x_res[:, m, ts(t, TS)],
                                    start=(m == 0),
                                    stop=(m == DCH - 1),
                                )
                            nc.vector.tensor_mul(
                                qh_sb[:, ts(t, TS)], psq, sqq_bc[:, ts(t, TS)]
                            )
                            psr = PQK.tile([128, TS], f32, tag="pqk")
                            for m in range(DCH):
                                nc.tensor.matmul(
                                    psr,
                                    lhsT=wqr_t[:, m, :],
                                    rhs=x_res[:, m, ts(t, TS)],
                                    start=(m == 0),
                                    stop=(m == DCH - 1),
                                )
                            tsw = TMPP.tile([128, TS], f32, tag="tmp")
                            nc.vector.stream_shuffle(tsw, psr, SWAP_MASK)
                            t1 = TMPP.tile([128, TS], f32, tag="tmp")
                            nc.vector.tensor_mul(t1, psr, ctabp[:, ts(t, TS)])
                            t2 = TMPP.tile([128, TS], f32, tag="tmp")
                            nc.vector.tensor_mul(t2, tsw, stabp[:, ts(t, TS)])
                            nc.vector.tensor_add(qrh_sb[:, ts(t, TS)], t1, t2)

                        for j in range(NT):
                            n_s = 4 * (j + 1)
                            LAG = 5
                            pctx = PCT.tile([128, TS], f32, tag="pct")
                            esum = ESP.tile([128, TS], bf, tag="esum")
                            e_ts = [None] * n_s

                            def consume(s):
                                nc.tensor.matmul(
                                    pctx,
                                    lhsT=v_sb[:, s, :],
                                    rhs=e_ts[s],
                                    start=(s == 0),
                                    stop=(s == n_s - 1),
                                )

                            for s in range(n_s):
                                pss = PSC.tile([128, TS], f32, tag="psc")
                                nc.tensor.matmul(
                                    pss,
                                    lhsT=kabs_sb[:, s * 128:(s + 1) * 128],
                                    rhs=qh_sb[:, ts(j, TS)],
                                    start=True,
                                    stop=False,
                                )
                                nc.tensor.matmul(
                                    pss,
                                    lhsT=k_sb[:, s * 128:(s + 1) * 128],
                                    rhs=qrh_sb[:, ts(j, TS)],
                                    start=False,
                                    stop=True,
                                )
                                e_t = EP.tile([128, TS], bf, tag="e")
                                nc.scalar.activation(e_t, pss, AF.Exp, scale=SM_SCALE)
                                if s >= 4 * j:
                                    nc.vector.tensor_mul(
                                        e_t, e_t, masks_sb[:, s - 4 * j, :]
                                    )
                                e_ts[s] = e_t
                                # softmax denominator: accumulate E on DVE
                                if s == 0:
                                    nc.vector.tensor_copy(esum, e_t)
                                else:
                                    nc.vector.tensor_add(esum, esum, e_t)
                                if s >= LAG:
                                    consume(s - LAG)
                            for s in range(max(0, n_s - LAG), n_s):
                                consume(s)
                            psz = PZ3.tile([1, TS], f32, tag="pz3")
                            nc.tensor.matmul(
                                psz, lhsT=ones_sb, rhs=esum,
                                start=True, stop=True,
                            )

                            zrow = ZR.tile([1, TS], f32, tag="zrow")
                            nc.vector.tensor_copy(zrow, psz)
                            zinv = ZR.tile([1, TS], f32, tag="zrow")
                            nc.vector.reciprocal(zinv, zrow)
                            zbc = TMPP.tile([128, TS], f32, tag="zbc")
                            nc.gpsimd.partition_broadcast(zbc, zinv[0:1, :])
                            nc.vector.tensor_mul(ctx_sb[h][j], pctx, zbc)

                # ---------------- phase 4: output projection -----------------
                with (
                    tc.tile_pool(name="P4", bufs=6) as P4,
                    tc.tile_pool(name="PP4", bufs=3, space="PSUM") as PP4,
                ):
                    for e in range(DCH):
                        for t in range(NT):
                            ps = PP4.tile([128, TS], f32, tag="pp4")
                            for q in range(HPG):
                                nc.tensor.matmul(
                                    ps,
                                    lhsT=wout_sb[:, q, e * 128:(e + 1) * 128],
                                    rhs=ctx_sb[q][t],
                                    start=(q == 0),
                                    stop=(q == HPG - 1),
                                )
                            o_t = P4.tile([128, TS], f32, tag="ot")
                            nc.vector.tensor_copy(o_t, ps)
                            nc.sync.dma_start(
                                out=outT.ap()[
                                    e * 128:(e + 1) * 128, ts(t, TS)
                                ],
                                in_=o_t,
                            )

    nc.compile()
    return nc


def _get_program():
    if "nc" not in _CACHE:
        _CACHE["nc"] = _build_program()
    return _CACHE["nc"]


def _chunk_pm(block):
    """[D?, 128cols] weight block -> partition-major [128, D//128, 128]."""
    d = block.shape[0]
    return np.ascontiguousarray(
        block.reshape(d // 128, 128, block.shape[1]).transpose(1, 0, 2)
    )


def _host_prep(inputs):
    """Fold weights on the host and build the 8 per-core input maps."""
    x = np.asarray(inputs["x"], np.float32)
    Wdq = np.asarray(inputs["Wdq"], np.float32)
    qw = np.asarray(inputs["q_norm_w"], np.float32)
    Wuq = np.asarray(inputs["Wuq"], np.float32) * qw[None, :]
    Wqr = np.asarray(inputs["Wqr"], np.float32) * qw[None, :]
    Wdkv = np.asarray(inputs["Wdkv"], np.float32)
    kvw = np.asarray(inputs["kv_norm_w"], np.float32)
    Wuk = np.asarray(inputs["Wuk"], np.float32) * kvw[None, :]
    Wuv = np.asarray(inputs["Wuv"], np.float32) * kvw[None, :]
    Wkr = np.asarray(inputs["Wkr"], np.float32)
    Wout = np.asarray(inputs["Wout"], np.float32)

    inv = 1.0 / (10000.0 ** (np.arange(0, HD, 2, dtype=np.float32) / HD))
    f = np.arange(T, dtype=np.float32)[None, :] * inv[:, None]   # [64, T]
    cosT, sinT = np.cos(f), np.sin(f)
    Ctab = np.repeat(cosT, 2, axis=0)                            # [128, T]
    Stab = np.repeat(sinT, 2, axis=0)
    Stab[0::2, :] *= -1.0                                        # pair-swap sign

    fH = np.arange(KVH, dtype=np.float32)[None, :] * inv[:, None]  # [64, KVH]
    cosH, sinH = np.cos(fH), np.sin(fH)

    def bft(a):
        return np.ascontiguousarray(a).astype(BF16)

    wdqT = Wdq.T            # [D, QL]
    wdkvT = Wdkv.T          # [D, KVL]
    # local-quarter stream: ql chunks first (gather starts earliest), then c
    wloc_np = np.stack(
        [_chunk_pm(wdqT[:, m * 128:(m + 1) * 128]) for m in range(QLCH)]
        + [_chunk_pm(wdkvT[:, m * 128:(m + 1) * 128]) for m in range(CCH)]
    )  # [M_LOC, 128, DCH, 128]
    wloc_b = bft(wloc_np)
    wuk_pm = bft(_chunk_pm(Wuk.T))                  # [128, CCH, HD]
    ctab_b = bft(Ctab)
    stab_b = bft(Stab)

    in_maps = []
    for b in range(B):
        x_T = bft(x[b].T)
        for g in range(G):
            # fold K-rope (fixed rotation per kv-head index) into Wkr
            Wkr_g = Wkr[g * HD:(g + 1) * HD, :]
            we, wo = Wkr_g[0::2, :], Wkr_g[1::2, :]
            c_g, s_g = cosH[:, g][:, None], sinH[:, g][:, None]
            Wkr_eff = np.empty_like(Wkr_g)
            Wkr_eff[0::2, :] = we * c_g - wo * s_g
            Wkr_eff[1::2, :] = we * s_g + wo * c_g

            # absorb Wdq: Q = (Wuq.Wdq) @ x * s_t (rms scale commutes out)
            Wuq_g = (Wuq[g * HPG * HD:(g + 1) * HPG * HD] @ Wdq).T  # [D, 512]
            Wqr_g = (Wqr[g * HPG * HD:(g + 1) * HPG * HD] @ Wdq).T
            wuq_np = np.stack(
                [_chunk_pm(Wuq_g[:, h * HD:(h + 1) * HD]) for h in range(HPG)]
            )
            wqr_np = np.stack(
                [_chunk_pm(Wqr_g[:, h * HD:(h + 1) * HD]) for h in range(HPG)]
            )

            in_maps.append(
                dict(
                    x_T=x_T,
                    xL_T=bft(x[b, g * TS:(g + 1) * TS, :].T),
                    wloc=wloc_b,
                    wkr=bft(_chunk_pm(Wkr_eff.T)),
                    wuq=bft(wuq_np),
                    wqr=bft(wqr_np),
                    wuk=wuk_pm,
                    wuv=bft(_chunk_pm(Wuv[g * HD:(g + 1) * HD].T)),
                    wout=bft(_chunk_pm(Wout[:, g * HPG * HD:(g + 1) * HPG * HD].T)),
                    ctab=ctab_b,
                    stab=stab_b,
                )
            )
    return in_maps


def kernel(**inputs):
    global LAST_RESULTS
    from concourse import bass_utils

    nc = _get_program()
    in_maps = _host_prep(inputs)
    res = bass_utils.run_bass_kernel_spmd(
        nc, in_maps, core_ids=list(range(NCORES))
    )
    LAST_RESULTS = res
    out = np.zeros((B, T, D), np.float32)
    for i, r in enumerate(res.results):
        out[i // G] += r["out_T"].T
    return out
